# revision 27
# baseline (speedup 1.0000x reference)
"""Trainium2 Bass kernel for nn_MultiHeadAttention (B=4, L=S=2048, D=1024, H=16, causal).

Sharding: 8 cores = 4 batches x 2 head-groups (8 heads each).
Per core: project its batch's q/k/v against its group's weight slices,
causal attention for 8 heads, output-projection against Wo column slice.
Host sums the 2 partial outputs per batch (tensor-parallel reduce).

v5 schedule (the attention phase is ACT(exp)-bound; everything else is
woven into its PE idle slots):
- upfront: V projection, QT[0] projection, K[0] projection (double-
  buffered PSUM via a shared pool tag -> no drain stalls).
- pair p attention: scores/ctx software-pipelined one block apart; the
  projections for pair p+1 (Q and K) are emitted as filler between
  blocks, filling the PE gap left by the exp latency. For pair 3 the
  filler is the Wo output projection of already-normalized chunks, so
  almost no PE-bound tail runs inside the worst HAM-throttle window.
- softmax normalization happens per chunk: fast DVE drains release the
  ctx PSUM banks, then the reciprocal/broadcast chain runs off the
  critical path.
- v2 tricks kept: row-sums folded into the ctx matmul via a ones column
  in V (65-row PSUM); causal diagonal blocks compute only surviving
  columns; all inputs host-pretransposed for straight DMAs.

All matmuls in bf16 with fp32 PSUM accumulation.
"""

import sys

if "/opt/trn_rl_repo" not in sys.path:
    sys.path.insert(0, "/opt/trn_rl_repo")

import numpy as np
import ml_dtypes

BF16 = ml_dtypes.bfloat16

# Problem constants (hardcoded per harness contract)
B, L, D, H = 4, 2048, 1024, 16
HD = D // H              # 64
NCORES = 8
GROUPS = 2               # head-groups (tensor parallel)
HG = H // GROUPS         # 8 heads per group
DG = HG * HD             # 512 out-dim per group

FULL_CFG = dict(T=L, DM=D, DG=DG)


def emit_mha(tc, aps, cfg):
    """Emit the per-core MHA program into TileContext tc."""
    import concourse.bass as bass
    from concourse import mybir

    nc = tc.nc
    f32 = mybir.dt.float32
    bf16 = mybir.dt.bfloat16
    Exp = mybir.ActivationFunctionType.Exp

    T, DM, DG_ = cfg["T"], cfg["DM"], cfg["DG"]
    TB = 128                  # s/l block
    LCH = min(512, T)         # l-chunk (moving-dim)
    nDch = DM // 128          # D chunks (contraction)
    nTt = T // TB             # token tiles
    nLch = T // LCH           # l-chunks
    nDiag = LCH // TB         # diagonal sub-blocks per chunk
    nPair = DG_ // 128        # head pairs (2 heads of 64 per pair)
    OCH = min(512, DM)        # Wo output chunk
    nOch = DM // OCH          # output chunks for Wo
    SCALE = 1.0 / np.sqrt(HD)

    import contextlib

    ctx = contextlib.ExitStack()
    with ctx:
        wpool = ctx.enter_context(tc.tile_pool(name="wts", bufs=1))
        xt_pool = ctx.enter_context(tc.tile_pool(name="xt", bufs=2 * nDch))
        qkv_pool = ctx.enter_context(tc.tile_pool(name="qkv", bufs=1))
        pt_pool = ctx.enter_context(tc.tile_pool(name="pt", bufs=4))
        ctxt_pool = ctx.enter_context(tc.tile_pool(name="ctxt", bufs=1))
        small = ctx.enter_context(tc.tile_pool(name="small", bufs=2))
        outsb_pool = ctx.enter_context(tc.tile_pool(name="outsb", bufs=2))
        kt_pool = ctx.enter_context(tc.tile_pool(name="ktp", bufs=2))
        cpc_pool = ctx.enter_context(tc.tile_pool(name="cpc", bufs=4))
        tiny = ctx.enter_context(tc.tile_pool(name="tiny", bufs=1))
        # PSUM budget (8 banks): "st" 2x2 (scores + V/Q projection) +
        # "kw" 1x2 (K proj / QT fillers / Wo, interleaved into attention) +
        # ctx 1x2
        st_ps = ctx.enter_context(tc.tile_pool(name="st_ps", bufs=2, space="PSUM"))
        kw_ps = ctx.enter_context(tc.tile_pool(name="kw_ps", bufs=2, space="PSUM"))
        ctx_ps_pool = ctx.enter_context(tc.tile_pool(name="ctx_ps", bufs=2, space="PSUM"))

        # ---- straight DMAs of host-pretransposed inputs ----
        def wload(nm):
            tiles = []
            for c in range(nDch):
                t = wpool.tile([128, DG_], bf16, tag=f"{nm}{c}")
                nc.sync.dma_start(out=t[:], in_=aps[nm][c])
                tiles.append(t)
            return tiles

        def xload(nm):
            tiles = []
            for c in range(nDch):
                t = xt_pool.tile([128, T], bf16, tag="xt")
                nc.sync.dma_start(out=t[:], in_=aps[nm][c])
                tiles.append(t)
            return tiles

        # sync-queue order = consumption order
        wvT = wload("wv")
        vT = xload("xv")
        mask2 = wpool.tile([128, 2, TB], bf16, tag="mask2")
        nc.sync.dma_start(out=mask2[:, 0, :], in_=aps["maskt"][:])
        nc.sync.dma_start(out=mask2[:, 1, :], in_=aps["maskt"][:])
        wqT = wload("wq")
        qT = xload("xq")
        wkT = wload("wk")
        kT = xload("xk")
        woT = []
        for c in range(DG_ // 128):
            t = wpool.tile([128, DM], bf16, tag=f"woT{c}")
            nc.sync.dma_start(out=t[:], in_=aps["wo"][c])
            woT.append(t)

        # ---- V projection: V[st] [128, 8, 65] (s on partitions; per-head 64
        # value dims + a ones column: the ctx matmul's 65th output row then
        # accumulates the softmax denominator for free) ----
        V = []
        for st in range(nTt):
            ps = st_ps.tile([128, HG, HD], f32, tag="st")
            for c in range(nDch):
                nc.tensor.matmul(ps[:, :, :], lhsT=vT[c][:, st * TB:(st + 1) * TB],
                                 rhs=wvT[c][:], start=(c == 0), stop=(c == nDch - 1))
            vt = qkv_pool.tile([128, HG, HD + 1], bf16, tag=f"V{st}")
            nc.vector.tensor_copy(vt[:, :, 0:HD], ps[:, :, :])
            nc.vector.memset(vt[:, :, HD:HD + 1], 1.0)
            V.append(vt)

        QT = [[None] * nLch for _ in range(nPair)]
        KT = [[None] * nLch for _ in range(nPair)]

        def emit_qt(m, n, half, ps_box):
            """Half a QT[m][n] projection (4 of 8 K-chunks)."""
            if half == 0:
                pool = st_ps if ps_box[1] else kw_ps
                tag = "st" if ps_box[1] else "kw"
                ps_box[0] = pool.tile([128, LCH], f32, tag=tag, name=f"qtps{m}_{n}")
            ps = ps_box[0]
            for c in range(half * nDch // 2, (half + 1) * nDch // 2):
                nc.tensor.matmul(ps[:], lhsT=wqT[c][:, m * 128:(m + 1) * 128],
                                 rhs=qT[c][:, n * LCH:(n + 1) * LCH],
                                 start=(c == 0), stop=(c == nDch - 1))
            if half == 1:
                qtn = qkv_pool.tile([128, LCH], bf16, tag=f"QT{m}_{n}", name=f"QT{m}_{n}")
                nc.vector.tensor_copy(qtn[:], ps[:])
                QT[m][n] = qtn

        def emit_kt(p, n, half, ps_box):
            """Half a KT[p][n] projection."""
            if half == 0:
                ps_box[0] = kw_ps.tile([128, LCH], f32, tag="kw", name=f"ktps{p}_{n}")
            ps = ps_box[0]
            for c in range(half * nDch // 2, (half + 1) * nDch // 2):
                nc.tensor.matmul(ps[:], lhsT=wkT[c][:, p * 128:(p + 1) * 128],
                                 rhs=kT[c][:, n * LCH:(n + 1) * LCH],
                                 start=(c == 0), stop=(c == nDch - 1))
            if half == 1:
                kt_t = kt_pool.tile([128, LCH], bf16, tag=f"KT{n}", name=f"KT{n}_{p}")
                nc.vector.tensor_copy(kt_t[:], ps[:])
                KT[p][n] = kt_t

        # upfront (inside the HAM free burst): QT[0], KT[0] double-buffered
        for n in range(nLch):
            box = [None, True]
            emit_qt(0, n, 0, box)
            emit_qt(0, n, 1, box)
        for n in range(nLch):
            box = [None]
            emit_kt(0, n, 0, box)
            emit_kt(0, n, 1, box)

        def proj_fillers(pnext):
            """Filler units projecting QT/KT for the next pair."""
            units = []
            for n in range(nLch):
                qbox = [None, False]
                units.append(lambda n=n, b=qbox: emit_qt(pnext, n, 0, b))
                units.append(lambda n=n, b=qbox: emit_qt(pnext, n, 1, b))
            for n in range(nLch):
                kbox = [None]
                units.append(lambda n=n, b=kbox: emit_kt(pnext, n, 0, b))
                units.append(lambda n=n, b=kbox: emit_kt(pnext, n, 1, b))
            return units

        ctxT = [[None] * nLch for _ in range(nPair)]
        osb_box = {}

        def emit_wo(lt, oc):
            """One Wo output block: y[lt, oc*OCH:] = ctx(lt) @ woT[:, oc]."""
            if oc == 0:
                osb_box[lt] = outsb_pool.tile([128, DM], bf16, tag="osb", name=f"osb{lt}")
            osb = osb_box[lt]
            ps = kw_ps.tile([128, OCH], f32, tag="kw")
            for dc in range(nPair):
                lhsT = ctxT[dc][lt // nDiag][:, (lt % nDiag) * TB:(lt % nDiag) * TB + TB]
                nc.tensor.matmul(ps[:], lhsT=lhsT,
                                 rhs=woT[dc][:, oc * OCH:(oc + 1) * OCH],
                                 start=(dc == 0), stop=(dc == nPair - 1))
            nc.vector.tensor_copy(osb[:, oc * OCH:(oc + 1) * OCH], ps[:])
            if oc == nOch - 1:
                nc.sync.dma_start(out=aps["y"][lt * TB:(lt + 1) * TB, :], in_=osb[:])

        for p in range(nPair):
            fillers = proj_fillers(p + 1) if p + 1 < nPair else []
            fi = 0
            qts = QT[p]
            ktn = KT[p]
            for i in range(nLch):
                nsb = (i + 1) * nDiag
                cps_a = ctx_ps_pool.tile([HD + 1, LCH], f32, tag="ctx")
                cps_b = ctx_ps_pool.tile([HD + 1, LCH], f32, tag="ctx")

                def emit_ctx(j, pt, co):
                    st = (j == 0)
                    en = (j == nsb - 1)
                    nc.tensor.matmul(cps_a[:, co:], lhsT=V[j][:, 2 * p, :],
                                     rhs=pt[:, 0, co:], start=st, stop=en,
                                     skip_group_check=True)
                    nc.tensor.matmul(cps_b[:, co:], lhsT=V[j][:, 2 * p + 1, :],
                                     rhs=pt[:, 1, co:], start=st, stop=en,
                                     skip_group_check=True)

                pending = None
                for j in range(nsb):
                    ktj = ktn[j // nDiag]
                    koff = (j % nDiag) * TB
                    r = j - nDiag * i
                    co = r * TB if r >= 0 else 0   # first surviving l column
                    sp = st_ps.tile([128, 2, LCH], f32, tag="st")
                    nc.tensor.matmul(sp[:, 0, co:],
                                     lhsT=ktj[0:64, koff:koff + TB],
                                     rhs=qts[i][0:64, co:],
                                     start=True, stop=True)
                    nc.tensor.matmul(sp[:, 1, co:],
                                     lhsT=ktj[64:128, koff:koff + TB],
                                     rhs=qts[i][64:128, co:],
                                     start=True, stop=True)
                    pt = pt_pool.tile([128, 2, LCH], bf16, tag="pt")
                    nc.scalar.activation(pt[:, :, co:], sp[:, :, co:], Exp, scale=float(SCALE))
                    if r >= 0:
                        nc.vector.tensor_mul(pt[:, :, co:co + TB], pt[:, :, co:co + TB],
                                             mask2[:, :, :])
                    # ctx runs one block behind scores so the PE never waits
                    # on the exp; filler (next pair's projections / Wo) soaks
                    # up the remaining ACT-bound slack.
                    if pending is not None:
                        emit_ctx(*pending)
                    pending = (j, pt, co)
                    if fi < len(fillers):
                        fillers[fi]()
                        fi += 1
                emit_ctx(*pending)
                # ---- per-chunk softmax normalization ----
                # drains release the ctx PSUM banks fast (the release gates
                # the next chunk's first ctx matmul, bufs=2); the
                # recip/broadcast chain then runs off the critical path.
                cpc_a = cpc_pool.tile([HD + 1, LCH], f32, tag="cpc")
                nc.vector.tensor_copy(cpc_a[:], cps_a[:])
                cpc_b = cpc_pool.tile([HD + 1, LCH], f32, tag="cpc")
                nc.vector.tensor_copy(cpc_b[:], cps_b[:])
                srec = small.tile([1, 2 * LCH], f32, tag="srec")
                nc.gpsimd.dma_start(out=srec[0:1, 0:LCH], in_=cpc_a[HD:HD + 1, :])
                nc.gpsimd.dma_start(out=srec[0:1, LCH:2 * LCH], in_=cpc_b[HD:HD + 1, :])
                rec01 = tiny.tile([1, 2 * LCH], f32, tag="rec01")
                nc.vector.reciprocal(rec01[:], srec[:])
                rb = tiny.tile([128, 2 * LCH], f32, tag="rb")
                nc.gpsimd.partition_broadcast(rb[:], rec01[0:1, :])
                ct = ctxt_pool.tile([128, LCH], bf16, tag=f"ctxT{p}_{i}")
                nc.vector.tensor_mul(ct[0:64, :], cpc_a[0:64, :], rb[0:64, 0:LCH])
                # rb is partition-broadcast: rows 0:64 match 64:128, keeping
                # both SBUF inputs at base partition 0.
                nc.vector.tensor_mul(ct[64:128, :], cpc_b[0:64, :], rb[0:64, LCH:2 * LCH])
                ctxT[p][i] = ct
                if p == nPair - 1 and i + 1 < nLch:
                    # pair 3: chunk i is fully normalized -> its Wo blocks
                    # become filler for chunk i+1's attention
                    for lt in range(i * nDiag, (i + 1) * nDiag):
                        for oc in range(nOch):
                            fillers.append(lambda lt=lt, oc=oc: emit_wo(lt, oc))
            while fi < len(fillers):
                fillers[fi]()
                fi += 1

        # Wo for the final chunk
        for lt in range((nLch - 1) * nDiag, nTt):
            for oc in range(nOch):
                emit_wo(lt, oc)


def build_nc(cfg):
    """Build and compile the per-core Bass program."""
    import concourse.bacc as bacc
    import concourse.tile as tile
    from concourse import mybir

    T, DM, DG_ = cfg["T"], cfg["DM"], cfg["DG"]

    nc = bacc.Bacc("TRN2", target_bir_lowering=False, debug=False)
    f32 = mybir.dt.float32
    bf16 = mybir.dt.bfloat16
    aps = {}
    specs = [
        ("maskt", [128, 128], bf16),
        ("wo", [DG_ // 128, 128, DM], bf16),
    ]
    for x in ("xq", "xk", "xv"):
        specs.append((x, [DM // 128, 128, T], bf16))
    for w in ("wq", "wk", "wv"):
        specs.append((w, [DM // 128, 128, DG_], bf16))
    for nm, shape, dt in specs:
        aps[nm] = nc.dram_tensor(nm, shape, dt, kind="ExternalInput").ap()
    aps["y"] = nc.dram_tensor("y", [T, DM], bf16, kind="ExternalOutput").ap()

    with tile.TileContext(nc) as tc:
        emit_mha(tc, aps, cfg)
    nc.compile()
    return nc


_CACHE = {}


def _get_nc():
    if "nc" not in _CACHE:
        _CACHE["nc"] = build_nc(FULL_CFG)
    return _CACHE["nc"]


def _chunks(arr_t):
    """[D, N] -> [D/128, 128, N] bf16 chunk-major layout."""
    d, n = arr_t.shape
    return np.ascontiguousarray(arr_t.reshape(d // 128, 128, n).astype(BF16))


def shard_inputs(q, k, v, Wq, Wk, Wv, Wo):
    """Build the per-core input maps (8 cores = 4 batches x 2 groups)."""
    maskt = np.triu(np.ones((128, 128), dtype=np.float32)).astype(BF16)
    xs = {}
    for b in range(B):
        xs[("xq", b)] = _chunks(q[b].T)
        xs[("xk", b)] = _chunks(k[b].T)
        xs[("xv", b)] = _chunks(v[b].T)
    ws = {}
    for g in range(GROUPS):
        rows = slice(g * DG, (g + 1) * DG)
        ws[("wq", g)] = _chunks(Wq[rows].T)
        ws[("wk", g)] = _chunks(Wk[rows].T)
        ws[("wv", g)] = _chunks(Wv[rows].T)
        ws[("wo", g)] = _chunks(Wo[:, rows].T)
    in_maps = []
    for core in range(NCORES):
        b, g = divmod(core, GROUPS)
        m = {"maskt": maskt, "wo": ws[("wo", g)]}
        for x in ("xq", "xk", "xv"):
            m[x] = xs[(x, b)]
        for w in ("wq", "wk", "wv"):
            m[w] = ws[(w, g)]
        in_maps.append(m)
    return in_maps


def kernel(q, k, v, mask, Wq, Wk, Wv, Wo):
    from concourse import bass_utils

    q = np.asarray(q, dtype=np.float32)
    k = np.asarray(k, dtype=np.float32)
    v = np.asarray(v, dtype=np.float32)
    Wq = np.asarray(Wq, dtype=np.float32)
    Wk = np.asarray(Wk, dtype=np.float32)
    Wv = np.asarray(Wv, dtype=np.float32)
    Wo = np.asarray(Wo, dtype=np.float32)

    nc = _get_nc()
    in_maps = shard_inputs(q, k, v, Wq, Wk, Wv, Wo)
    res = bass_utils.run_bass_kernel_spmd(nc, in_maps, core_ids=list(range(NCORES)))
    out = np.zeros((B, L, D), dtype=np.float32)
    for core in range(NCORES):
        b = core // GROUPS
        out[b] += res.results[core]["y"].astype(np.float32)
    return out
